# revision 3
# baseline (speedup 1.0000x reference)
"""Trainium2 Bass kernel for CrossAttention.

Reference computation (per batch item b):
    xt = x[b].reshape(C, N).T            # [N, C] tokens
    q = xt @ Wq.T + bq ; k = yt @ Wk.T + bk ; v = yt @ Wv.T + bv
    out = softmax(q @ k.T) @ v           # [N, C]
    return out.T.reshape(C, H, W)

Sharding: data-parallel over batch B=8 across the 8 NeuronCores (one batch
item per core). Each core holds the full 256x256 projection weights.

Device-side scheme (per core), all matmuls in float32r (full-rate PE mode):
  - x, y natively [C, N]; QT = WqT.T @ x + bq, KT = WkT.T @ y (bk dropped: a
    per-query-row additive constant q.bk in every score cancels in softmax).
  - V computed in [N, C] layout with a ones column appended -> PV matmul
    emits the softmax denominator as column 256 for free.
  - scores computed directly transposed: sT[kv,q] = KT_chunk.T @ QT_block,
    exp'd (no max subtraction; |scores| <~ 35 so fp32 exp is safe) straight
    from PSUM into probsT[kv,q] tiles = exactly the PV matmul's lhsT.
  - out[q, 0:256] / out[q, 256] normalizes; bv added after the final PE
    transpose (partition dim = c there, so it is a per-partition bias).
  - final [q,c] -> [c,q] PE transposes make the output DMA contiguous in
    the required [C, N] layout.
"""

import numpy as np

import concourse.bass as bass
import concourse.mybir as mybir
import concourse.tile as tile
from concourse import bacc
from concourse.bass_utils import run_bass_kernel_spmd
from concourse.masks import make_identity

B, C, H, W = 8, 256, 48, 48
NTOK = H * W  # 2304
N_CORES = 8

DT = mybir.dt.float32
DTR = mybir.dt.float32r
FP = mybir.ActivationFunctionType


def build_program(ntok=NTOK, q_super=256):
    """Build the per-core SPMD Bass program."""
    nkv = ntok // 128          # kv chunks of 128 tokens
    nqs = ntok // q_super      # query super-blocks
    nqq = q_super // 128       # 128-wide query sub-blocks per super-block
    n_half = 2                 # C=256 -> two 128-partition halves

    nc = bacc.Bacc("TRN2", target_bir_lowering=False, debug=False,
                   num_devices=N_CORES)

    x_d = nc.dram_tensor("x", [C, ntok], DTR, kind="ExternalInput").ap()
    y_d = nc.dram_tensor("y", [C, ntok], DTR, kind="ExternalInput").ap()
    wqt_d = nc.dram_tensor("wqt", [C, C], DTR, kind="ExternalInput").ap()
    wkt_d = nc.dram_tensor("wkt", [C, C], DTR, kind="ExternalInput").ap()
    wvt_d = nc.dram_tensor("wvt", [C, C], DTR, kind="ExternalInput").ap()
    bq_d = nc.dram_tensor("bq", [C], DT, kind="ExternalInput").ap()
    bv_d = nc.dram_tensor("bv", [C], DT, kind="ExternalInput").ap()
    vones_d = nc.dram_tensor("vones", [128, 2 * nkv], DTR,
                             kind="ExternalInput").ap()
    out_d = nc.dram_tensor("out", [C, ntok], DT, kind="ExternalOutput").ap()

    with tile.TileContext(nc) as tc:
        with (
            tc.tile_pool(name="const", bufs=1) as constp,
            tc.tile_pool(name="xy", bufs=1) as xyp,
            tc.tile_pool(name="qk", bufs=1) as qkp,
            tc.tile_pool(name="vw", bufs=1) as vwp,
            tc.tile_pool(name="probs", bufs=2) as probsp,
            tc.tile_pool(name="epi", bufs=3) as epip,
            tc.tile_pool(name="ps_a", bufs=3, space="PSUM") as ps_a,
            tc.tile_pool(name="ps_pv", bufs=2, space="PSUM") as ps_pv,
            tc.tile_pool(name="ps_tr", bufs=2, space="PSUM") as ps_tr,
        ):
            ident = constp.tile([128, 128], DT)
            make_identity(nc, ident[:])

            x_t = xyp.tile([128, n_half, ntok], DTR, tag="x")
            y_t = xyp.tile([128, n_half, ntok], DTR, tag="y")
            nc.sync.dma_start(x_t[:], x_d.rearrange("(kh p) n -> p kh n", p=128))
            nc.sync.dma_start(y_t[:], y_d.rearrange("(kh p) n -> p kh n", p=128))

            wq_t = constp.tile([128, n_half, C], DTR, tag="wq")
            wk_t = constp.tile([128, n_half, C], DTR, tag="wk")
            wv_t = constp.tile([128, n_half, C], DTR, tag="wv")
            nc.sync.dma_start(wq_t[:], wqt_d.rearrange("(kh p) n -> p kh n", p=128))
            nc.sync.dma_start(wk_t[:], wkt_d.rearrange("(kh p) n -> p kh n", p=128))
            nc.sync.dma_start(wv_t[:], wvt_d.rearrange("(kh p) n -> p kh n", p=128))
            bq_t = constp.tile([128, n_half], DT, tag="bq")
            bv_t = constp.tile([128, n_half], DT, tag="bv")
            nc.sync.dma_start(bq_t[:], bq_d.rearrange("(kh p) -> p kh", p=128))
            nc.sync.dma_start(bv_t[:], bv_d.rearrange("(kh p) -> p kh", p=128))

            # ---- projections: QT[c, n] and KT[c, n] ----
            qt_t = qkp.tile([128, n_half, ntok], DTR, tag="qt")
            kt_t = qkp.tile([128, n_half, ntok], DTR, tag="kt")
            for (w_t, src, dst, bias) in (
                (wq_t, x_t, qt_t, bq_t),
                (wk_t, y_t, kt_t, None),
            ):
                for cc in range(n_half):
                    for n0 in range(0, ntok, 512):
                        nw = min(512, ntok - n0)
                        ps = ps_a.tile([128, 512], DT, tag="ps_a")
                        for kh in range(n_half):
                            nc.tensor.matmul(
                                ps[:, :nw],
                                w_t[:, kh, cc * 128:(cc + 1) * 128],
                                src[:, kh, n0:n0 + nw],
                                start=(kh == 0), stop=(kh == n_half - 1),
                            )
                        if bias is not None:
                            nc.scalar.activation(
                                dst[:, cc, n0:n0 + nw], ps[:, :nw], FP.Identity,
                                bias=bias[:, cc:cc + 1],
                            )
                        else:
                            nc.scalar.activation(
                                dst[:, cc, n0:n0 + nw], ps[:, :nw], FP.Copy)

            # ---- projection V in [n, c] layout, with ones column ----
            v_t = vwp.tile([128, nkv, C + 2], DTR, tag="v")
            nc.sync.dma_start(
                v_t[:, :, C:C + 2],
                vones_d.rearrange("p (j o) -> p j o", o=2))
            for j in range(nkv):
                ps = ps_a.tile([128, 512], DT, tag="ps_a")
                for kh in range(n_half):
                    nc.tensor.matmul(
                        ps[:, :C],
                        y_t[:, kh, j * 128:(j + 1) * 128],
                        wv_t[:, kh, :],
                        start=(kh == 0), stop=(kh == n_half - 1),
                    )
                nc.vector.tensor_copy(v_t[:, j, 0:C], ps[:, :C])

            # ---- attention ----
            for qs in range(nqs):
                q0 = qs * q_super
                pbt = probsp.tile([128, nkv, q_super], DTR, tag="pbt")
                for j in range(nkv):
                    ps = ps_a.tile([128, 512], DT, tag="ps_a")
                    for kh in range(n_half):
                        nc.tensor.matmul(
                            ps[:, :q_super],
                            kt_t[:, kh, j * 128:(j + 1) * 128],
                            qt_t[:, kh, q0:q0 + q_super],
                            start=(kh == 0), stop=(kh == n_half - 1),
                        )
                    nc.scalar.activation(pbt[:, j, :], ps[:, :q_super], FP.Exp)

                for qq in range(nqq):
                    po = ps_pv.tile([128, C + 2], DT, tag="po")
                    for j in range(nkv):
                        nc.tensor.matmul(
                            po[:],
                            pbt[:, j, qq * 128:(qq + 1) * 128],
                            v_t[:, j, :],
                            start=(j == 0), stop=(j == nkv - 1),
                        )
                    r_t = epip.tile([128, 1], DT, tag="r")
                    nc.vector.reciprocal(r_t[:], po[:, C:C + 1])
                    o_sb = epip.tile([128, C], DT, tag="osb")
                    nc.scalar.activation(o_sb[:], po[:, 0:C], FP.Copy,
                                         scale=r_t[:])
                    ot_sb = epip.tile([128, n_half, 128], DT, tag="otsb")
                    for cc in range(n_half):
                        pt = ps_tr.tile([128, 128], DT, tag="pt")
                        nc.tensor.transpose(
                            pt[:], o_sb[:, cc * 128:(cc + 1) * 128], ident[:])
                        nc.vector.tensor_scalar_add(
                            ot_sb[:, cc, :], pt[:], bv_t[:, cc:cc + 1])
                    nq0 = q0 + qq * 128
                    nc.sync.dma_start(
                        out_d.rearrange("(cc p) n -> p cc n", p=128)
                        [:, :, nq0:nq0 + 128],
                        ot_sb[:],
                    )

    nc.compile()
    return nc


_CACHE = {}


def _get_program(ntok=NTOK):
    key = ntok
    if key not in _CACHE:
        _CACHE[key] = build_program(ntok=ntok)
    return _CACHE[key]


def kernel(x, y, Wq, bq, Wk, bk, Wv, bv):
    x = np.ascontiguousarray(np.asarray(x, dtype=np.float32))
    y = np.ascontiguousarray(np.asarray(y, dtype=np.float32))
    Wq = np.asarray(Wq, dtype=np.float32)
    Wk = np.asarray(Wk, dtype=np.float32)
    Wv = np.asarray(Wv, dtype=np.float32)
    bq = np.ascontiguousarray(np.asarray(bq, dtype=np.float32))
    bv = np.ascontiguousarray(np.asarray(bv, dtype=np.float32))

    b, c, h, w = x.shape
    ntok = h * w
    wqt = np.ascontiguousarray(Wq.T)
    wkt = np.ascontiguousarray(Wk.T)
    wvt = np.ascontiguousarray(Wv.T)

    nc = _get_program(ntok)
    vones = np.ones((128, 2 * (ntok // 128)), dtype=np.float32)
    in_maps = []
    for i in range(N_CORES):
        in_maps.append({
            "x": x[i].reshape(c, ntok),
            "y": y[i].reshape(c, ntok),
            "wqt": wqt, "wkt": wkt, "wvt": wvt,
            "bq": bq, "bv": bv, "vones": vones,
        })
    res = run_bass_kernel_spmd(nc, in_maps, list(range(N_CORES)))
    out = np.empty((b, c, h, w), dtype=np.float32)
    for i in range(N_CORES):
        out[i] = res.results[i]["out"].reshape(c, h, w)
    return out
